# revision 1
# baseline (speedup 1.0000x reference)
"""Trainium2 Bass kernel for nn_AdaptiveConv2 (dense CNN + per-pixel adaptive conv).

Data-parallel over batch: 8 images -> 8 NeuronCores, no collectives.

Per-core plan (channel-major bf16 convs on PE, pixel-major epilogue on DVE):
  - 7 conv-bn-tanh layers: BN folded into weights/bias on host. Activations
    live in SBUF padded [128, 130, 130] buffers: partitions 0-63 hold the
    64 channels, partitions 64-127 hold the same channels shifted right by
    one pixel. A 3x3 conv then needs only 6 PE matmuls per 512-pixel tile
    (3 with K=128 pairing two kernel taps, 3 with K=64), N=512, M=128
    (out channels duplicated so ACT can write both the base copy and the
    shifted copy from PSUM partitions 0-63 / 64-127).
  - Epilogue: out[c,m,p] = sum_k bfeat[m*6+k,p] * g[k,c,p] where
    g[k] = depthwise 3x3 conv of x with basis kernel k (computed on PE as
    block-diagonal matmuls, 2 k's per 128-wide M, bases folded in on host).
    g and bfeat are DMA-transposed to pixel-major; per 128-pixel row DVE
    does 3 broadcast tensor_tensor products (m x c x k-pair, last dim
    packed for the 2x mode) and a 3-op pairwise tree add; the result is
    transposed back to channel-major on PE (via identity matmul), cast to
    f32 by the ACT copy, and DMA'd out with a channel-interleaved (c*6+m)
    scatter AP.
  - Emission follows a row-wavefront (shifted per-layer windows, 1-wave
    layer skew) so PE/ACT/DVE/DMA overlap; cost-model timeline ~481us/core.
"""

import os
import sys

sys.path.insert(0, "/opt/trn_rl_repo")

# The kernel executes through the axon PJRT backend; a harness that pins
# JAX_PLATFORMS=cpu (common for reference runs) would hide the NeuronCores.
if os.environ.get("JAX_PLATFORMS") and "axon" not in os.environ["JAX_PLATFORMS"]:
    if "jax" not in sys.modules:
        del os.environ["JAX_PLATFORMS"]

import numpy as np
import ml_dtypes

BF16 = ml_dtypes.bfloat16
EPS = 1e-5

C = 64
H = W = 128
PW = 130  # padded width/height
PADN = PW * PW
NPIX = H * W
NT = 32  # tiles of 4 rows (512 pixels)
FEAT = 6
NB = 6
OC = FEAT * NB  # 36
NCONV_COLS = 7 * 6 * 128
NG_COLS = 3 * 6 * 128
NCOLS = NCONV_COLS + NG_COLS

_CACHE = {}


def _build_graph():
    import concourse.bacc as bacc
    import concourse.bass as bass
    import concourse.tile as tile
    import concourse.mybir as mybir
    from contextlib import ExitStack

    f32 = mybir.dt.float32
    bf16 = mybir.dt.bfloat16

    nc = bacc.Bacc("TRN2", target_bir_lowering=False, debug=False, num_devices=8)

    xd_ext = nc.dram_tensor("xd", [128, PADN], bf16, kind="ExternalInput").ap()
    wts_ext = nc.dram_tensor("wts", [128, NCOLS], bf16, kind="ExternalInput").ap()
    bias_ext = nc.dram_tensor("bias", [128, 8], f32, kind="ExternalInput").ap()
    out_ext = nc.dram_tensor("out", [C * NB, NPIX], f32, kind="ExternalOutput").ap()

    Tanh = mybir.ActivationFunctionType.Tanh
    MULT = mybir.AluOpType.mult
    ADD = mybir.AluOpType.add

    ctx = ExitStack()
    with tile.TileContext(nc) as tc, ctx:
        singles = ctx.enter_context(tc.tile_pool(name="singles", bufs=1))
        cpsum = ctx.enter_context(tc.tile_pool(name="cpsum", bufs=4, space="PSUM"))
        gpsum = ctx.enter_context(tc.tile_pool(name="gpsum", bufs=2, space="PSUM"))
        tpsum = ctx.enter_context(tc.tile_pool(name="tpsum", bufs=2, space="PSUM"))
        gcm_pool = ctx.enter_context(tc.tile_pool(name="gcm", bufs=6))
        gt_pool = ctx.enter_context(tc.tile_pool(name="gt", bufs=5))
        bft_pool = ctx.enter_context(tc.tile_pool(name="bft", bufs=6))
        acc_pool = ctx.enter_context(tc.tile_pool(name="acc", bufs=8))
        ofl_pool = ctx.enter_context(tc.tile_pool(name="ofl", bufs=4))

        x_t = singles.tile([128, PW, PW], bf16)
        actA = singles.tile([128, PW, PW], bf16)
        actB = singles.tile([128, PW, PW], bf16)
        bfeat = singles.tile([64, H, W], bf16)
        wtile = singles.tile([128, NCOLS], bf16)
        btile = singles.tile([128, 8], f32)
        ident = singles.tile([128, 128], bf16)
        from concourse.masks import make_identity

        make_identity(nc, ident)

        # input DMAs (L0 weights first; x in 10 chunks so early tiles land first)
        nc.sync.dma_start(out=wtile[:, 0:768], in_=wts_ext[:, 0:768])
        nc.sync.dma_start(out=btile, in_=bias_ext)
        xd3 = xd_ext.rearrange("p (a b) -> p a b", a=10)
        x_t_flat = x_t.rearrange("p a b -> p (a b)").rearrange(
            "p (a b) -> p a b", a=10
        )
        for ch in range(10):
            nc.gpsimd.dma_start(out=x_t_flat[:, ch, :], in_=xd3[:, ch, :])
        for li in range(1, 7):
            nc.sync.dma_start(
                out=wtile[:, li * 768 : (li + 1) * 768],
                in_=wts_ext[:, li * 768 : (li + 1) * 768],
            )
        nc.sync.dma_start(
            out=wtile[:, NCONV_COLS:], in_=wts_ext[:, NCONV_COLS:]
        )

        # bfeat rows 36-63 are only read (transposed) as padding — zero once
        nc.vector.memset(bfeat[32:64, :, :], 0.0)
        # zero only the halos (interior gets overwritten; halo must stay 0)
        for buf in (actA, actB):
            nc.vector.memset(buf[:, 0, :], 0.0)       # top halo row
            nc.vector.memset(buf[:, PW - 1, :], 0.0)  # bottom halo row
            nc.vector.memset(buf[:, 1 : PW - 1, 0:2], 0.0)   # left halo + dup col
            nc.vector.memset(buf[:, 1 : PW - 1, PW - 1 :], 0.0)  # right halo

        layer_in = [x_t, actA, actB, actA, actB, actA, actB]
        layer_out = [actA, actB, actA, actB, actA, actB, bfeat]

        def conv_rows(li, t):
            # layer li's tile t covers output rows [4t-li, 4t+3-li] (clipped):
            # the shifted window makes every cross-layer dependency point to
            # tiles t-1/t, so a 1-wave layer skew suffices.
            r0 = max(0, 4 * t - li)
            r1 = min(H - 1, 4 * t + 3 - li)
            return r0, r1

        def emit_conv(li, t):
            r0, r1 = conv_rows(li, t)
            if r1 < r0:
                return
            nr = r1 - r0 + 1
            src = layer_in[li]
            dst = layer_out[li]
            M = 128 if li < 6 else OC
            ps = cpsum.tile([128, 512], f32, tag="cps")
            for issue in range(6):
                di = (issue % 3) - 1
                dj = 0 if issue < 3 else 1
                K = 128 if issue < 3 else 64
                col0 = (li * 6 + issue) * 128
                nc.tensor.matmul(
                    ps[:M, : nr * 128],
                    wtile[0:K, col0 : col0 + M],
                    src[0:K, r0 + 1 + di : r0 + 1 + di + nr, 1 + dj : 129 + dj],
                    start=(issue == 0),
                    stop=(issue == 5),
                )
            ps3 = ps.rearrange("p (a b) -> p a b", b=128)
            if li < 6:
                nc.scalar.activation(
                    dst[0:64, r0 + 1 : r0 + 1 + nr, 1:129],
                    ps3[0:64, 0:nr],
                    Tanh,
                    bias=btile[0:64, li : li + 1],
                )
                nc.scalar.activation(
                    dst[64:128, r0 + 1 : r0 + 1 + nr, 2:130],
                    ps3[64:128, 0:nr],
                    Tanh,
                    bias=btile[64:128, li : li + 1],
                )
            else:
                nc.scalar.activation(
                    dst[0:OC, r0 : r0 + nr, :],
                    ps3[0:OC, 0:nr],
                    Tanh,
                    bias=btile[0:OC, li : li + 1],
                )

        def emit_bft(t):
            r0, r1 = conv_rows(6, t)
            nr = r1 - r0 + 1
            bft = bft_pool.tile([128, 4, 64], bf16, tag="bft")
            nc.sync.dma_start_transpose(
                bft[:, 0:nr, :], bfeat[0:64, r0 : r0 + nr, :]
            )
            return bft

        def emit_g(t):
            r0, r1 = conv_rows(6, t)
            nr = r1 - r0 + 1
            gt4 = gt_pool.tile([128, 4, 384], bf16, tag="gt4")
            for kp in range(3):
                ps = gpsum.tile([128, 512], f32, tag="gps")
                for issue in range(6):
                    di = (issue % 3) - 1
                    dj = 0 if issue < 3 else 1
                    K = 128 if issue < 3 else 64
                    col0 = NCONV_COLS + (kp * 6 + issue) * 128
                    nc.tensor.matmul(
                        ps[:, : nr * 128],
                        wtile[0:K, col0 : col0 + 128],
                        x_t[0:K, r0 + 1 + di : r0 + 1 + di + nr, 1 + dj : 129 + dj],
                        start=(issue == 0),
                        stop=(issue == 5),
                    )
                gcm = gcm_pool.tile([128, 512], bf16, tag="gcm")
                nc.scalar.copy(gcm[:, : nr * 128], ps[:, : nr * 128])
                nc.sync.dma_start_transpose(
                    gt4[:, 0:nr, kp * 128 : (kp + 1) * 128], gcm[:, : nr * 128]
                )
            return gt4

        def emit_epilogue(t, gt4, bft):
            r0, r1 = conv_rows(6, t)
            for rl in range(r1 - r0 + 1):
                # P_kp[p, (m,c,kk)] = bfeat[p, m*6+2kp+kk] * g[p, kp-chunk c*2+kk]
                # Last AP dim stays packed (stride 1, count 2) on all operands
                # so the tensor_tensors run in the 2x DVE mode. (Walrus caps
                # DVE free APs at 3 dims, hence one product per k-pair.)
                prod = acc_pool.tile([128, 3, 6, 64, 2], bf16, tag="prod", bufs=2)
                for kp in range(3):
                    g_in = bass.AP(
                        tensor=gt4.tensor,
                        offset=gt4.offset + rl * 384 + kp * 128,
                        ap=[gt4.ap[0], [0, 6], [2, 64], [1, 2]],
                    )
                    b_in = bass.AP(
                        tensor=bft.tensor,
                        offset=bft.offset + rl * 64 + 2 * kp,
                        ap=[bft.ap[0], [6, 6], [0, 64], [1, 2]],
                    )
                    nc.vector.tensor_tensor(prod[:, kp], g_in, b_in, MULT)
                # tree-reduce the 6 k values (pairwise, kk stays packed)
                d = acc_pool.tile([128, 384, 2], bf16, tag="dsum", bufs=2)
                pk = prod.rearrange("p q m c b -> p q (m c) b")
                nc.vector.tensor_tensor(d, pk[:, 0], pk[:, 1], ADD)
                nc.vector.tensor_tensor(d, d, pk[:, 2], ADD)
                # final add writes acc directly in output channel order
                # ch = c*6 + m (strided non-last dims are free; this op is
                # 1x-mode regardless), so the store needs one DMA per row.
                acc = acc_pool.tile([128, 384], bf16, tag="acc")
                din0 = bass.AP(tensor=d.tensor, offset=d.offset,
                               ap=[d.ap[0], [128, 6], [2, 64]])
                din1 = bass.AP(tensor=d.tensor, offset=d.offset + 1,
                               ap=[d.ap[0], [128, 6], [2, 64]])
                aout = bass.AP(tensor=acc.tensor, offset=acc.offset,
                               ap=[acc.ap[0], [1, 6], [6, 64]])
                nc.vector.tensor_tensor(aout, din0, din1, ADD)
                # transpose back to channel-major on PE, cast to f32 on ACT
                tps = tpsum.tile([128, 384], bf16, tag="tps")
                for j in range(3):
                    nc.tensor.transpose(
                        tps[:, j * 128 : (j + 1) * 128],
                        acc[:, j * 128 : (j + 1) * 128],
                        ident,
                    )
                ofl = ofl_pool.tile([128, 384], f32, tag="ofl")
                nc.scalar.copy(ofl, tps)
                # store: transposed rows are already channel-ordered
                row = r0 + rl
                dst = bass.AP(
                    tensor=out_ext.tensor,
                    offset=row * 128,
                    ap=[[NPIX, 128], [128 * NPIX, 3], [1, 128]],
                )
                nc.sync.dma_start(out=dst, in_=ofl)

        # Wave w: layer li handles tile (w - li). With the shifted windows the
        # halo dependencies point to tiles t-1/t of the previous layer, both
        # emitted in earlier waves, so no same-wave serial chain forms.
        def valid(li, t):
            r0, r1 = conv_rows(li, t)
            return r1 >= r0

        gt4_tiles = {}
        bft_tiles = {}
        NTT = 34  # tiles 0..33 cover every layer's shifted windows
        for w in range(NTT + 9):
            for li in range(7):
                t = w - li
                if 0 <= t < NTT and valid(li, t):
                    emit_conv(li, t)
                    if li == 6:
                        bft_tiles[t] = emit_bft(t)
            tg = w - 6
            if 0 <= tg < NTT and valid(6, tg):
                gt4_tiles[tg] = emit_g(tg)
            te = w - 8
            if 0 <= te < NTT and valid(6, te):
                emit_epilogue(te, gt4_tiles.pop(te), bft_tiles.pop(te))

    nc.compile()
    return nc


def _fold_bn(w, b, g, be, m, v):
    scale = g / np.sqrt(v + EPS)
    wf = w * scale[:, None, None, None]
    bf = (b - m) * scale + be
    return wf.astype(np.float32), bf.astype(np.float32)


def _prep_weights(w0, b0, g0, be0, m0, v0, wm, bm, gm, bem, mm, vm,
                  wl, bl, gl, bel, ml, vl, bases):
    wts = np.zeros((128, NCOLS), np.float32)
    bias = np.zeros((128, 8), np.float32)
    layers = [(w0, b0, g0, be0, m0, v0)]
    for i in range(5):
        layers.append((wm[i], bm[i], gm[i], bem[i], mm[i], vm[i]))
    layers.append((wl, bl, gl, bel, ml, vl))
    for li, (w, b, g, be, m, v) in enumerate(layers):
        wf, bf = _fold_bn(w, b, g, be, m, v)
        oc = wf.shape[0]
        for issue in range(6):
            di = (issue % 3) - 1
            col0 = (li * 6 + issue) * 128
            if issue < 3:
                # pair: rows 0:64 tap (di, 0); rows 64:128 tap (di, -1)
                wts[0:64, col0 : col0 + oc] = wf[:, :, di + 1, 1].T
                wts[64:128, col0 : col0 + oc] = wf[:, :, di + 1, 0].T
                if oc == 64:
                    wts[0:64, col0 + 64 : col0 + 128] = wf[:, :, di + 1, 1].T
                    wts[64:128, col0 + 64 : col0 + 128] = wf[:, :, di + 1, 0].T
            else:
                wts[0:64, col0 : col0 + oc] = wf[:, :, di + 1, 2].T
                if oc == 64:
                    wts[0:64, col0 + 64 : col0 + 128] = wf[:, :, di + 1, 2].T
        bias[0:oc, li] = bf
        if oc == 64:
            bias[64:128, li] = bf
    eye = np.eye(64, dtype=np.float32)
    for kp in range(3):
        for issue in range(6):
            di = (issue % 3) - 1
            col0 = NCONV_COLS + (kp * 6 + issue) * 128
            for kk in range(2):
                k = 2 * kp + kk
                # g output column layout: c*2 + kk (keeps the epilogue
                # tensor_tensor's last AP dim packed)
                sl = slice(col0 + kk, col0 + 128, 2)
                if issue < 3:
                    wts[0:64, sl] = eye * bases[k, (di + 1) * 3 + 1]
                    wts[64:128, sl] = eye * bases[k, (di + 1) * 3 + 0]
                else:
                    wts[0:64, sl] = eye * bases[k, (di + 1) * 3 + 2]
    return wts.astype(BF16), bias


def _prep_x(xn):
    xp = np.zeros((C, PW, PW), np.float32)
    xp[:, 1:129, 1:129] = xn
    flat = xp.reshape(C, PADN)
    dup = np.zeros((128, PADN), np.float32)
    dup[0:64] = flat
    dup[64:128, 1:] = flat[:, :-1]
    return dup.astype(BF16)


def get_nc():
    if "nc" not in _CACHE:
        _CACHE["nc"] = _build_graph()
    return _CACHE["nc"]


def kernel(**inputs):
    from concourse.bass_utils import run_bass_kernel_spmd

    nc = get_nc()
    x = np.asarray(inputs["x"], np.float32)
    wts, bias = _prep_weights(
        *[np.asarray(inputs[k], np.float32) for k in
          ("w0", "b0", "g0", "be0", "m0", "v0", "wm", "bm", "gm", "bem",
           "mm", "vm", "wl", "bl", "gl", "bel", "ml", "vl", "bases")]
    )
    in_maps = [
        {"xd": _prep_x(x[n]), "wts": wts, "bias": bias} for n in range(8)
    ]
    res = run_bass_kernel_spmd(nc, in_maps, core_ids=list(range(8)))
    out = np.stack([r["out"] for r in res.results])
    return out.reshape(8, C * NB, H, W).astype(np.float32)


if __name__ == "__main__":
    rng = np.random.default_rng(0)
    ins = {
        "x": rng.standard_normal((8, C, H, W), dtype=np.float32),
        "w0": rng.standard_normal((64, 64, 3, 3), dtype=np.float32) * 0.05,
        "b0": rng.standard_normal(64, dtype=np.float32) * 0.05,
        "g0": rng.random(64, dtype=np.float32),
        "be0": rng.standard_normal(64, dtype=np.float32) * 0.05,
        "m0": rng.standard_normal(64, dtype=np.float32) * 0.05,
        "v0": rng.random(64, dtype=np.float32),
        "wm": rng.standard_normal((5, 64, 64, 3, 3), dtype=np.float32) * 0.05,
        "bm": rng.standard_normal((5, 64), dtype=np.float32) * 0.05,
        "gm": rng.random((5, 64), dtype=np.float32),
        "bem": rng.standard_normal((5, 64), dtype=np.float32) * 0.05,
        "mm": rng.standard_normal((5, 64), dtype=np.float32) * 0.05,
        "vm": rng.random((5, 64), dtype=np.float32),
        "wl": rng.standard_normal((36, 64, 3, 3), dtype=np.float32) * 0.05,
        "bl": rng.standard_normal(36, dtype=np.float32) * 0.05,
        "gl": rng.random(36, dtype=np.float32),
        "bel": rng.standard_normal(36, dtype=np.float32) * 0.05,
        "ml": rng.standard_normal(36, dtype=np.float32) * 0.05,
        "vl": rng.random(36, dtype=np.float32),
        "bases": rng.standard_normal((6, 9), dtype=np.float32),
    }
    out = kernel(**ins)
    print("out", out.shape, out.dtype, np.abs(out).mean())



# revision 2
# speedup vs baseline: 1.0984x; 1.0984x over previous
"""Trainium2 Bass kernel for nn_AdaptiveConv2 — v2 (pixel-major convs).

Data-parallel over batch: 8 images -> 8 NeuronCores, no collectives.

Cost-model-driven design (matmul cost = output free size; stationary loads
free):
  - Convs pixel-major: stationary = activation slices [K=(row-parity,ch),
    M=128 px of one image row], moving = weights [K, 64] -> N=64/matmul.
    7 matmuls per output row (6 tap-reads + 1 K=1 bias read). Activations
    stored channel-major in (row-parity, channel) partition layout
    [128, 66 row-pair groups, 130 cols]; one PE transpose per row-pair
    returns the pixel-major tanh output to that layout, one DVE copy per
    8 rows commits it.
  - g (depthwise basis conv of x): rank-1 im2col — x replicated x9 taps on
    partitions (host-prepped, DMA-windowed); 8 matmuls of N=48 per row
    cover all 9 taps at once. Lands pixel-major in PSUM; two strided
    copies (ACT+DVE) pack bf16 (kp, c, kk) for the epilogue.
  - Epilogue per row on DVE: 3 broadcast products (2x packed) + 3 tree
    adds; PE transposes to channel order, ACT/Pool cast f32, 1 DMA/row.
"""

import os
import sys

sys.path.insert(0, "/opt/trn_rl_repo")

if os.environ.get("JAX_PLATFORMS") and "axon" not in os.environ["JAX_PLATFORMS"]:
    if "jax" not in sys.modules:
        del os.environ["JAX_PLATFORMS"]

import numpy as np
import ml_dtypes

BF16 = ml_dtypes.bfloat16
EPS = 1e-5

C = 64
H = W = 128
NPIX = H * W
NG = 66        # row-pair groups incl top/bottom halo
PW = 130       # padded width
FEAT = 6
NB = 6
OC = FEAT * NB           # 36
NWC = 7 * 12 * 64        # conv weight cols
NWG = 8 * 48             # g weight cols
NGRP = 16                # 8-row groups
SKEW = 1                 # wave skew between conv layers (same-wave emission
                         # order satisfies the one-group-ahead dependency)
EPI_W = 8                # epilogue wave offset

_CACHE = {}


def _build_graph():
    import concourse.bacc as bacc
    import concourse.bass as bass
    import concourse.tile as tile
    import concourse.mybir as mybir
    from contextlib import ExitStack

    f32 = mybir.dt.float32
    bf16 = mybir.dt.bfloat16

    nc = bacc.Bacc("TRN2", target_bir_lowering=False, debug=False, num_devices=8)

    xd_ext = nc.dram_tensor("xd", [128, NG * PW], bf16, kind="ExternalInput").ap()
    xr_ext = nc.dram_tensor("xr", [72, 8 * 16 * 8 * PW], bf16,
                            kind="ExternalInput").ap()
    wc_ext = nc.dram_tensor("wc", [128, NWC], bf16, kind="ExternalInput").ap()
    wg_ext = nc.dram_tensor("wg", [72, NWG], bf16, kind="ExternalInput").ap()
    bias_ext = nc.dram_tensor("bias", [128, 448], bf16, kind="ExternalInput").ap()
    out_ext = nc.dram_tensor("out", [C * NB, NPIX], f32, kind="ExternalOutput").ap()

    Tanh = mybir.ActivationFunctionType.Tanh
    MULT = mybir.AluOpType.mult
    ADD = mybir.AluOpType.add

    ctx = ExitStack()
    with tile.TileContext(nc) as tc, ctx:
        singles = ctx.enter_context(tc.tile_pool(name="singles", bufs=1))
        cpsum = ctx.enter_context(tc.tile_pool(name="cpsum", bufs=3, space="PSUM"))
        tpsum = ctx.enter_context(tc.tile_pool(name="tpsum", bufs=2, space="PSUM"))
        gpsum = ctx.enter_context(tc.tile_pool(name="gpsum", bufs=2, space="PSUM"))
        opsum = ctx.enter_context(tc.tile_pool(name="opsum", bufs=1, space="PSUM"))
        pixp = ctx.enter_context(tc.tile_pool(name="pixp", bufs=3))
        gsb_pool = ctx.enter_context(tc.tile_pool(name="gsb", bufs=18))
        xrw_pool = ctx.enter_context(tc.tile_pool(name="xrw", bufs=3))
        acc_pool = ctx.enter_context(tc.tile_pool(name="acc", bufs=6))
        ofl_pool = ctx.enter_context(tc.tile_pool(name="ofl", bufs=4))

        x_t = singles.tile([128, NG, PW], bf16)
        actA = singles.tile([128, NG, PW], bf16)
        actB = singles.tile([128, NG, PW], bf16)
        bft = singles.tile([128, H, OC], bf16)
        wc = singles.tile([128, NWC], bf16)
        wg = singles.tile([72, NWG], bf16)
        btile = singles.tile([128, 448], bf16)
        ones_t = singles.tile([128, 128], bf16)
        ident = singles.tile([128, 128], bf16)
        from concourse.masks import make_identity

        make_identity(nc, ident)
        nc.vector.memset(ones_t[0:1, :], 1.0)

        # input DMAs: L0 weights first, then x in chunks so early groups land
        nc.sync.dma_start(out=wc[:, 0:768], in_=wc_ext[:, 0:768])
        nc.sync.dma_start(out=btile, in_=bias_ext)
        xd3 = xd_ext.rearrange("p (a b) -> p a b", a=6)
        x_t_flat = x_t.rearrange("p a b -> p (a b)").rearrange(
            "p (a b) -> p a b", a=6
        )
        for ch in range(6):
            nc.gpsimd.dma_start(out=x_t_flat[:, ch, :], in_=xd3[:, ch, :])
        for li in range(1, 7):
            nc.sync.dma_start(
                out=wc[:, li * 768 : (li + 1) * 768],
                in_=wc_ext[:, li * 768 : (li + 1) * 768],
            )
        nc.sync.dma_start(out=wg, in_=wg_ext)

        # zero halos once (copies only ever write G 1..64, cols 1:129)
        for buf in (actA, actB):
            nc.vector.memset(buf[:, 0, :], 0.0)
            nc.vector.memset(buf[:, NG - 1, :], 0.0)
            nc.vector.memset(buf[:, 1 : NG - 1, 0:1], 0.0)
            nc.vector.memset(buf[:, 1 : NG - 1, PW - 1 :], 0.0)

        layer_in = [x_t, actA, actB, actA, actB, actA, actB]
        layer_out = [actA, actB, actA, actB, actA, actB, None]

        xr4 = xr_ext.rearrange("p (s w q) -> p s w q", s=8, w=16)

        def emit_conv_group(li, t):
            """Layer li, rows 8t..8t+7 (one 8-row group)."""
            src = layer_in[li]
            M = 64 if li < 6 else OC
            ps = cpsum.tile([128, 512], f32, tag="cps")
            for j8 in range(8):
                r = 8 * t + j8
                a = r // 2
                off = M * j8
                if r % 2 == 0:
                    reads = [(a, 0), (a + 1, 3)]
                else:
                    reads = [(a + 1, 6), (a + 2, 9)]
                first = True
                for (G, b0) in reads:
                    for dj in range(3):
                        col0 = li * 768 + (b0 + dj) * 64
                        nc.tensor.matmul(
                            ps[:, off : off + M],
                            src[0:128, G, dj : dj + 128],
                            wc[0:128, col0 : col0 + M],
                            start=first,
                            stop=False,
                        )
                        first = False
                nc.tensor.matmul(
                    ps[:, off : off + M],
                    ones_t[0:1, 0:128],
                    btile[0:1, li * 64 : li * 64 + M],
                    start=False,
                    stop=True,
                )
            if li < 6:
                P = pixp.tile([128, 8, 64], bf16, tag="pixP")
                ps3 = ps.rearrange("p (a b) -> p a b", a=8)
                nc.scalar.activation(P, ps3, Tanh)
                dst = layer_out[li]
                T = tpsum.tile([128, 512], bf16, tag="tps")
                for j in range(4):
                    nc.tensor.transpose(
                        T[:, j * 128 : (j + 1) * 128],
                        P[:, 2 * j : 2 * j + 2, :],
                        ident,
                    )
                T4 = T.rearrange("p (a b) -> p a b", a=4)
                nc.scalar.copy(dst[:, 4 * t + 1 : 4 * t + 5, 1:129], T4)
            else:
                ps3 = bass.AP(
                    tensor=ps.tensor,
                    offset=ps.offset,
                    ap=[ps.ap[0], [OC, 8], [1, OC]],
                )
                nc.scalar.activation(bft[:, 8 * t : 8 * t + 8, :], ps3, Tanh)

        def emit_g_row(r, xrw):
            wr = r % 8
            gps = gpsum.tile([128, 384], f32, tag="gps")
            for s in range(8):
                nc.tensor.matmul(
                    gps[:, s * 48 : (s + 1) * 48],
                    xrw[0:72, s, wr, 1:129],
                    wg[0:72, s * 48 : (s + 1) * 48],
                    start=True,
                    stop=True,
                )
            gsb = gsb_pool.tile([128, 384], bf16, tag="gsb")
            # pack (s, kp, kk, c8) psum f32 -> (kp, c=8s+c8, kk) bf16
            for kk in range(2):
                src = bass.AP(
                    tensor=gps.tensor,
                    offset=gps.offset + kk * 8,
                    ap=[gps.ap[0], [48, 8], [16, 3], [1, 8]],
                )
                dst = bass.AP(
                    tensor=gsb.tensor,
                    offset=gsb.offset + kk,
                    ap=[gsb.ap[0], [16, 8], [128, 3], [2, 8]],
                )
                if kk == 0:
                    nc.scalar.copy(dst, src)
                else:
                    nc.vector.tensor_copy(dst, src)
            return gsb

        def emit_epilogue_row(r, gsb):
            # two rows per group run on Pool (SBUF-only ops) to offload DVE;
            # they get their own tile tags so a slow Pool row can't starve
            # the DVE rows' buffer rotation
            on_pool = r % 8 >= 6
            ve = nc.gpsimd if on_pool else nc.vector
            sfx = "p" if on_pool else ""
            prod = acc_pool.tile([128, 3, 6, 64, 2], bf16, tag="prod" + sfx,
                                 bufs=3)
            for kp in range(3):
                g_in = bass.AP(
                    tensor=gsb.tensor,
                    offset=gsb.offset + kp * 128,
                    ap=[gsb.ap[0], [0, 6], [2, 64], [1, 2]],
                )
                b_in = bass.AP(
                    tensor=bft.tensor,
                    offset=bft.offset + r * OC + 2 * kp,
                    ap=[bft.ap[0], [6, 6], [0, 64], [1, 2]],
                )
                ve.tensor_tensor(prod[:, kp], g_in, b_in, MULT)
            d = acc_pool.tile([128, 384, 2], bf16, tag="dsum" + sfx, bufs=3)
            pk = prod.rearrange("p q m c b -> p q (m c) b")
            ve.tensor_tensor(d, pk[:, 0], pk[:, 1], ADD)
            ve.tensor_tensor(d, d, pk[:, 2], ADD)
            acc = acc_pool.tile([128, 384], bf16, tag="acc" + sfx)
            din0 = bass.AP(tensor=d.tensor, offset=d.offset,
                           ap=[d.ap[0], [128, 6], [2, 64]])
            din1 = bass.AP(tensor=d.tensor, offset=d.offset + 1,
                           ap=[d.ap[0], [128, 6], [2, 64]])
            aout = bass.AP(tensor=acc.tensor, offset=acc.offset,
                           ap=[acc.ap[0], [1, 6], [6, 64]])
            ve.tensor_tensor(aout, din0, din1, ADD)
            tps = opsum.tile([128, 384], bf16, tag="ops")
            for j in range(3):
                nc.tensor.transpose(
                    tps[:, j * 128 : (j + 1) * 128],
                    acc[:, j * 128 : (j + 1) * 128],
                    ident,
                )
            ofl = ofl_pool.tile([128, 384], f32, tag="ofl")
            nc.scalar.copy(ofl, tps)
            dst = bass.AP(
                tensor=out_ext.tensor,
                offset=r * 128,
                ap=[[NPIX, 128], [128 * NPIX, 3], [1, 128]],
            )
            nc.sync.dma_start(out=dst, in_=ofl)

        # wavefront
        xrw_tiles = {}
        gsb_rows = {}
        NW = EPI_W + NGRP
        for w in range(NW):
            win = w - (EPI_W - 4)
            if 0 <= win < NGRP:
                xrw = xrw_pool.tile([72, 8, 8, PW], bf16, tag="xrw")
                xrwf = xrw.rearrange("p a b c -> p (a b c)")
                nc.sync.dma_start(out=xrwf, in_=xr4[:, :, win, :])
                xrw_tiles[win] = xrw
            for li in range(7):
                t = w - SKEW * li
                if 0 <= t < NGRP:
                    emit_conv_group(li, t)
            tg = w - (EPI_W - 1)
            if 0 <= tg < NGRP:
                for wr in range(8):
                    gsb_rows[8 * tg + wr] = emit_g_row(
                        8 * tg + wr, xrw_tiles[tg]
                    )
                xrw_tiles.pop(tg)
            te = w - EPI_W
            if 0 <= te < NGRP:
                for wr in range(8):
                    r = 8 * te + wr
                    emit_epilogue_row(r, gsb_rows.pop(r))

    nc.compile()
    return nc


def _fold_bn(w, b, g, be, m, v):
    scale = g / np.sqrt(v + EPS)
    wf = w * scale[:, None, None, None]
    bf = (b - m) * scale + be
    return wf.astype(np.float32), bf.astype(np.float32)


def _prep_weights(w0, b0, g0, be0, m0, v0, wm, bm, gm, bem, mm, vm,
                  wl, bl, gl, bel, ml, vl, bases):
    wc = np.zeros((128, NWC), np.float32)
    bias = np.zeros((128, 448), np.float32)
    layers = [(w0, b0, g0, be0, m0, v0)]
    for i in range(5):
        layers.append((wm[i], bm[i], gm[i], bem[i], mm[i], vm[i]))
    layers.append((wl, bl, gl, bel, ml, vl))
    for li, (w, b, g, be, m, v) in enumerate(layers):
        wf, bf = _fold_bn(w, b, g, be, m, v)
        oc = wf.shape[0]
        # blocks (64 cols each): A(dj): even 0 / odd w[.,.,0,dj];
        # B: w[1]/w[2]; C: w[0]/w[1]; D: w[2]/0   (even=rows 0:64)
        for dj in range(3):
            cA = li * 768 + (0 + dj) * 64
            cB = li * 768 + (3 + dj) * 64
            cC = li * 768 + (6 + dj) * 64
            cD = li * 768 + (9 + dj) * 64
            wc[64:128, cA : cA + oc] = wf[:, :, 0, dj].T
            wc[0:64, cB : cB + oc] = wf[:, :, 1, dj].T
            wc[64:128, cB : cB + oc] = wf[:, :, 2, dj].T
            wc[0:64, cC : cC + oc] = wf[:, :, 0, dj].T
            wc[64:128, cC : cC + oc] = wf[:, :, 1, dj].T
            wc[0:64, cD : cD + oc] = wf[:, :, 2, dj].T
        bias[0, li * 64 : li * 64 + oc] = bf
    # g weights: row (c8*9 + l), col s*48 + kp*16 + kk*8 + c8
    wgm = np.zeros((72, NWG), np.float32)
    for kp in range(3):
        for kk in range(2):
            k = 2 * kp + kk
            for c8 in range(8):
                for l in range(9):
                    wgm[c8 * 9 + l, kp * 16 + kk * 8 + c8 :: 48] = bases[k, l]
    return wc.astype(BF16), wgm.astype(BF16), bias.astype(BF16)


def _prep_x(xn):
    # xd: [128=(r01*64+c), 66, 130]; content x[c, 2G-2+r01, col-1]
    xd = np.zeros((128, NG, PW), np.float32)
    xp = np.zeros((C, 2 * NG + 1, PW), np.float32)  # rows -2..130
    xp[:, 2 : 2 + H, 1 : 1 + W] = xn
    for r01 in range(2):
        xd[r01 * 64 : r01 * 64 + 64] = xp[:, r01 : r01 + 2 * NG : 2, :]
    # xrep: [72=(c8*9+l), s, win, wr, col] = x[8s+c8, R+di, col+dj-1]
    # with R = 8*win+wr, l = 3*(di+1)+(dj+1)
    xq = np.zeros((C, H + 2, PW + 2), np.float32)  # rows -1..128, cols -2..130
    xq[:, 1 : 1 + H, 2 : 2 + W] = xn
    xrep = np.zeros((72, 8, 16, 8, PW), np.float32)
    for di in range(-1, 2):
        for dj in range(-1, 2):
            l = 3 * (di + 1) + (dj + 1)
            sl = xq[:, 1 + di : 1 + di + H, 1 + dj : 1 + dj + PW]
            for s in range(8):
                xrep[np.arange(8) * 9 + l, s] = sl[8 * s : 8 * s + 8].reshape(
                    8, 16, 8, PW
                )
    return (
        xd.reshape(128, NG * PW).astype(BF16),
        xrep.reshape(72, 8 * 16 * 8 * PW).astype(BF16),
    )


def get_nc():
    if "nc" not in _CACHE:
        _CACHE["nc"] = _build_graph()
    return _CACHE["nc"]


def kernel(**inputs):
    from concourse.bass_utils import run_bass_kernel_spmd

    nc = get_nc()
    x = np.asarray(inputs["x"], np.float32)
    wc, wgm, bias = _prep_weights(
        *[np.asarray(inputs[k], np.float32) for k in
          ("w0", "b0", "g0", "be0", "m0", "v0", "wm", "bm", "gm", "bem",
           "mm", "vm", "wl", "bl", "gl", "bel", "ml", "vl", "bases")]
    )
    in_maps = []
    for n in range(8):
        xd, xrep = _prep_x(x[n])
        in_maps.append({"xd": xd, "xr": xrep, "wc": wc, "wg": wgm,
                        "bias": bias})
    res = run_bass_kernel_spmd(nc, in_maps, core_ids=list(range(8)))
    out = np.stack([r["out"] for r in res.results])
    return out.reshape(8, C * NB, H, W).astype(np.float32)
